# revision 2
# baseline (speedup 1.0000x reference)
"""2D DCT-II (4096x4096) on 8 Trainium2 NeuronCores (axon/PJRT SPMD).

Math: the reference computes C = A_M @ x @ A_N^T where the Makhoul even-odd
permutation is folded into dense DFT-derived tables built from the expk
inputs (see _tables):
  amT[i, u]  = A_M[u, i] = 0.5*(eMr[u]*cos(2pi*u*pinv[i]/M) + eMi[u]*sin(..))
  annT[c, v] = A_N^T[c, v] = 2.0*(eNr[v]*cos(2pi*v*pinv[c]/N) + eNi[v]*sin(..))

Distribution (8 cores), chosen so no operand ever needs a transpose on the
host or on the device:
  core k holds x[:, cols_k] (column shard, natural layout).
  phase 1: ZT_k = (A_M @ x[:, cols_k])^T  via  matmul(lhsT=x_cols, rhs=amT)
           -> [512 v, 4096 u], written as 8 [512, 512] blocks (u-block j).
  AllToAll: block j of core k -> core j; after, core k holds
           Z^T[:, rows_k] = [4096 c, 512 u]  (block m = c rows 512m..).
  phase 2: C[rows_k, :] = matmul(lhsT=Z^T[:, rows_k], rhs=annT)
           -> [512 u, 4096 v] = the final row shard of C.
Host: x is uploaded column-sharded (bf16), C comes back row-sharded (bf16)
as one global jax array - zero host-side reshuffling. Tables are uploaded
once (replicated across cores) and cached on device.

Everything is bf16 except PSUM accumulation (fp32). Measured end-to-end
rel err ~4e-3 vs the f64 reference (gate is 2e-2).
"""
import numpy as np

_NCORES = 8
_SZ = 4096
_RPC = _SZ // _NCORES  # 512 rows/cols per core
_KT = _SZ // 128       # 32 contraction tiles
_UP = 1024             # streamed panel width (2KB DMA lines in bf16)

_state = {}


# --------------------------------------------------------------------------
# Bass kernel
# --------------------------------------------------------------------------
def _build_bass():
    import concourse.bacc as bacc
    import concourse.mybir as mybir
    from concourse.tile import TileContext

    fp32 = mybir.dt.float32
    bf16 = mybir.dt.bfloat16
    nc = bacc.Bacc("TRN2", target_bir_lowering=False, debug=False,
                   num_devices=_NCORES)
    xc = nc.declare_dram_parameter("xc", [_SZ, _RPC], bf16, isOutput=False)
    amT = nc.declare_dram_parameter("amT", [_SZ, _SZ], bf16, isOutput=False)
    annT = nc.declare_dram_parameter("annT", [_SZ, _SZ], bf16, isOutput=False)
    cout = nc.declare_dram_parameter("cout", [_RPC, _SZ], bf16, isOutput=True)

    w_send = nc.dram_tensor("w_send", [_NCORES, _RPC, _RPC], bf16)
    w_recv = nc.dram_tensor("w_recv", [_NCORES, _RPC, _RPC], bf16)

    with TileContext(nc) as tc:
        # ---------- phase 1: ZT[v, u] = sum_r xc[r, v] * amT[r, u] ----------
        # xc resident in SBUF as [128, (kt, v)]; amT streamed in u panels.
        with (
            tc.tile_pool(name="xcp", bufs=1) as xc_pool,
            tc.tile_pool(name="am", bufs=2) as am_pool,
            tc.tile_pool(name="ps1", bufs=8, space="PSUM") as ps1_pool,
            tc.tile_pool(name="ev1", bufs=8) as ev1_pool,
        ):
            xcs = xc_pool.tile([128, _KT * _RPC], bf16)  # 4 MB
            nc.sync.dma_start(
                out=xcs[:].rearrange("p (kt v) -> p kt v", kt=_KT),
                in_=xc[:].rearrange("(kt p) v -> p kt v", p=128))
            for up in range(_SZ // _UP):
                am = am_pool.tile([128, _KT * _UP], bf16, tag="am")  # 8 MB
                nc.sync.dma_start(
                    out=am[:].rearrange("p (kt u) -> p kt u", kt=_KT),
                    in_=amT[:, up * _UP:(up + 1) * _UP]
                    .rearrange("(kt p) u -> p kt u", p=128))
                for vt in range(_RPC // 128):
                    for uh in range(_UP // 512):
                        ps = ps1_pool.tile([128, 512], fp32, tag="ps")
                        for kt in range(_KT):
                            nc.tensor.matmul(
                                ps[:],
                                xcs[:, kt * _RPC + vt * 128:
                                       kt * _RPC + vt * 128 + 128],
                                am[:, kt * _UP + uh * 512:
                                      kt * _UP + (uh + 1) * 512],
                                start=(kt == 0), stop=(kt == _KT - 1))
                        ev = ev1_pool.tile([128, 512], bf16, tag="ev")
                        nc.vector.tensor_copy(ev[:], ps[:])
                        j = (up * _UP + uh * 512) // _RPC  # dest core
                        nc.sync.dma_start(
                            out=w_send[j, vt * 128:(vt + 1) * 128, :],
                            in_=ev[:])

        # ---------- exchange ----------
        nc.gpsimd.collective_compute(
            "AllToAll",
            mybir.AluOpType.bypass,
            ins=[w_send[:]],
            outs=[w_recv[:]],
            replica_groups=[list(range(_NCORES))],
        )

        # ---------- phase 2: C[u, v] = sum_c ZT[c, u] * annT[c, v] ----------
        # w_recv resident as [128, (kt, u)]: global c = kt*128 + p with
        # kt = j*4 + s, i.e. w_recv[j, s*128+p, u]; annT streamed in v panels.
        with (
            tc.tile_pool(name="wr", bufs=1) as wr_pool,
            tc.tile_pool(name="an", bufs=2) as an_pool,
            tc.tile_pool(name="ps2", bufs=8, space="PSUM") as ps2_pool,
            tc.tile_pool(name="ev2", bufs=8) as ev2_pool,
        ):
            wr = wr_pool.tile([128, _KT * _RPC], bf16)  # 4 MB
            nc.sync.dma_start(
                out=wr[:].rearrange("p (j s u) -> p j s u", j=_NCORES, s=4),
                in_=w_recv[:].rearrange("j (s p) u -> p j s u", p=128))
            for vp in range(_SZ // _UP):
                an = an_pool.tile([128, _KT * _UP], bf16, tag="an")  # 8 MB
                nc.sync.dma_start(
                    out=an[:].rearrange("p (kt v) -> p kt v", kt=_KT),
                    in_=annT[:, vp * _UP:(vp + 1) * _UP]
                    .rearrange("(kt p) v -> p kt v", p=128))
                for ut in range(_RPC // 128):
                    for vh in range(_UP // 512):
                        ps = ps2_pool.tile([128, 512], fp32, tag="ps2")
                        for kt in range(_KT):
                            nc.tensor.matmul(
                                ps[:],
                                wr[:, kt * _RPC + ut * 128:
                                      kt * _RPC + ut * 128 + 128],
                                an[:, kt * _UP + vh * 512:
                                      kt * _UP + (vh + 1) * 512],
                                start=(kt == 0), stop=(kt == _KT - 1))
                        ev = ev2_pool.tile([128, 512], bf16, tag="ev2")
                        nc.vector.tensor_copy(ev[:], ps[:])
                        nc.sync.dma_start(
                            out=cout[ut * 128:(ut + 1) * 128,
                                     vp * _UP + vh * 512:
                                     vp * _UP + (vh + 1) * 512],
                            in_=ev[:])

    nc.compile()
    return nc


# --------------------------------------------------------------------------
# PJRT SPMD runner (compile once, run many)
# --------------------------------------------------------------------------
def _build_runner(nc, n_cores):
    import jax
    import jax.numpy as jnp
    from jax.sharding import Mesh, PartitionSpec as P, NamedSharding
    from jax.experimental.shard_map import shard_map
    import concourse.mybir as mybir
    from concourse import bass2jax
    from concourse.bass2jax import _bass_exec_p, partition_id_tensor

    bass2jax.install_neuronx_cc_hook()
    partition_name = (nc.partition_id_tensor.name
                      if nc.partition_id_tensor else None)

    # global shapes + shardings per bass parameter
    param_spec = {
        "xc": ((_SZ, _SZ), P(None, "core")),     # column shard
        "amT": ((_SZ, _SZ), P()),                # replicated
        "annT": ((_SZ, _SZ), P()),               # replicated
    }

    in_names, out_names, out_avals = [], [], []
    for alloc in nc.m.functions[0].allocations:
        if not isinstance(alloc, mybir.MemoryLocationSet):
            continue
        name = alloc.memorylocations[0].name
        if alloc.kind == "ExternalInput":
            if name != partition_name:
                in_names.append(name)
        elif alloc.kind == "ExternalOutput":
            shape = tuple(alloc.tensor_shape)
            dtype = mybir.dt.np(alloc.dtype)
            out_names.append(name)
            out_avals.append(jax.core.ShapedArray(shape, dtype))
    n_params = len(in_names)
    n_outs = len(out_avals)
    in_names_all = list(in_names) + out_names
    if partition_name is not None:
        in_names_all = in_names_all + [partition_name]
    donate = tuple(range(n_params, n_params + n_outs))

    def _body(*args):
        operands = list(args)
        if partition_name is not None:
            operands.append(partition_id_tensor())
        outs = _bass_exec_p.bind(
            *operands,
            out_avals=tuple(out_avals),
            in_names=tuple(in_names_all),
            out_names=tuple(out_names),
            lowering_input_output_aliases=(),
            sim_require_finite=True,
            sim_require_nnan=True,
            nc=nc,
        )
        return tuple(outs)

    devices = jax.devices()[:n_cores]
    mesh = Mesh(np.asarray(devices), ("core",))
    in_specs = tuple(param_spec[nm][1] for nm in in_names)
    out_sharding_specs = (P("core"),) * n_outs
    sharded = jax.jit(
        shard_map(_body, mesh=mesh,
                  in_specs=in_specs + out_sharding_specs,
                  out_specs=out_sharding_specs,
                  check_rep=False),
        donate_argnums=donate, keep_unused=True)

    x_shard = NamedSharding(mesh, P(None, "core"))
    rep_shard = NamedSharding(mesh, P())
    out_shard = NamedSharding(mesh, P("core"))
    _dev_cache = {}

    _zero_shapes = [(n_cores * a.shape[0], *a.shape[1:]) for a in out_avals]
    _zero_dtypes = [a.dtype for a in out_avals]
    _make_zeros = jax.jit(
        lambda: tuple(jnp.zeros(s, d)
                      for s, d in zip(_zero_shapes, _zero_dtypes)),
        out_shardings=(out_shard,) * len(_zero_shapes))

    def _put(name, arr):
        import jax as _jax
        sh = x_shard if name == "xc" else rep_shard
        darr = _jax.device_put(arr, sh)
        return darr

    def run(in_map, cache_names=(), block=True):
        """in_map: full global arrays keyed by bass param name."""
        import jax as _jax
        concat_in = []
        for name in in_names:
            if name in cache_names and name in _dev_cache:
                concat_in.append(_dev_cache[name])
                continue
            darr = _put(name, in_map[name])
            if name in cache_names:
                _jax.block_until_ready(darr)
                _dev_cache[name] = darr
            concat_in.append(darr)
        raw = sharded(*concat_in, *_make_zeros())
        if block:
            _jax.block_until_ready(raw)
        return raw[0] if n_outs == 1 else raw

    def bench(L):
        """Dispatch L back-to-back executions on cached inputs, block once.
        Returns elapsed wall seconds. Device executions serialize, so
        (bench(L2)-bench(L1))/(L2-L1) isolates per-call HW exec time from
        the constant dispatch overhead."""
        import time as _time
        import jax as _jax
        concat_in = [_dev_cache[name] for name in in_names]
        t0 = _time.perf_counter()
        outs = []
        for _ in range(L):
            outs.append(sharded(*concat_in, *_make_zeros()))
        _jax.block_until_ready(outs)
        return _time.perf_counter() - t0

    run.dev_cache = _dev_cache
    run.bench = bench
    run.mesh = mesh
    run.x_shard = x_shard
    return run


# --------------------------------------------------------------------------
# host-side tables
# --------------------------------------------------------------------------
def _tables(expkM, expkN):
    import ml_dtypes
    key = (expkM.tobytes(), expkN.tobytes())
    cached = _state.get("tables")
    if cached is not None and cached[0] == key:
        return cached[1], cached[2]
    run = _state.get("run")
    if run is not None:
        run.dev_cache.clear()
    n = _SZ
    i = np.arange(n)
    pm = np.where(i < (n + 1) // 2, 2 * i, 2 * (n - i) - 1)
    pinv = np.empty(n, dtype=np.int64)
    pinv[pm] = i
    # Cp[j, v] = cos(2pi * pinv[j] * v / n)
    ang = (2.0 * np.pi / n) * np.outer(pinv.astype(np.float64),
                                       i.astype(np.float64))
    Cp = np.cos(ang)
    Sp = np.sin(ang)
    eMr = expkM[:, 0].astype(np.float64)
    eMi = expkM[:, 1].astype(np.float64)
    eNr = expkN[:, 0].astype(np.float64)
    eNi = expkN[:, 1].astype(np.float64)
    bf16 = ml_dtypes.bfloat16
    annT = np.ascontiguousarray(
        (2.0 * (Cp * eNr[None, :] + Sp * eNi[None, :])).astype(bf16))
    amT = np.ascontiguousarray(
        (0.5 * (Cp * eMr[None, :] + Sp * eMi[None, :])).astype(bf16))
    _state["tables"] = (key, annT, amT)
    return annT, amT


def kernel(x, expkM, expkN, M, N):
    import ml_dtypes
    x = np.asarray(x, dtype=np.float32)
    expkM = np.asarray(expkM, dtype=np.float32)
    expkN = np.asarray(expkN, dtype=np.float32)
    assert x.shape == (_SZ, _SZ)

    annT, amT = _tables(expkM, expkN)
    if "run" not in _state:
        _state["run"] = _build_runner(_build_bass(), _NCORES)
    run = _state["run"]

    xb = x.astype(ml_dtypes.bfloat16)
    out = run({"xc": xb, "annT": annT, "amT": amT},
              cache_names=("annT", "amT"))
    return np.asarray(out).astype(np.float32)


# revision 12
# speedup vs baseline: 3.6620x; 3.6620x over previous
"""2D DCT-II (4096x4096) on 8 Trainium2 NeuronCores (axon/PJRT SPMD).

Math: the reference computes C = A_M @ x @ A_N^T where the Makhoul even-odd
permutation is folded into dense DFT-derived tables built from the expk
inputs (see _tables):
  amT[i, u]  = A_M[u, i] = 0.5*(eMr[u]*cos(2pi*u*pinv[i]/M) + eMi[u]*sin(..))
  annT[c, v] = A_N^T[c, v] = 2.0*(eNr[v]*cos(2pi*v*pinv[c]/N) + eNi[v]*sin(..))

Distribution (8 cores), chosen so no operand ever needs a transpose on the
host or on the device:
  core k holds x[:, cols_k] (column shard, natural layout).
  phase 1: ZT_k = (A_M @ x[:, cols_k])^T  via  matmul(lhsT=x_cols, rhs=amT)
           -> [512 v, 4096 u], written as 8 [512, 512] blocks (u-block j).
  AllToAll: block j of core k -> core j; after, core k holds
           Z^T[:, rows_k] = [4096 c, 512 u]  (block m = c rows 512m..).
  phase 2: C[rows_k, :] = matmul(lhsT=Z^T[:, rows_k], rhs=annT)
           -> [512 u, 4096 v] = the final row shard of C.
Host: x is uploaded column-sharded (bf16), C comes back row-sharded (bf16)
as one global jax array - zero host-side reshuffling. Tables are uploaded
once (replicated across cores) and cached on device.

Everything is bf16 except PSUM accumulation (fp32). Measured end-to-end
rel err ~4e-3 vs the f64 reference (gate is 2e-2).
"""
import numpy as np

_NCORES = 8
_SZ = 4096
_RPC = _SZ // _NCORES  # 512 rows/cols per core
_KT = _SZ // 128       # 32 contraction tiles
_UP = 1024             # streamed panel width (2KB DMA lines in bf16)

_state = {}


# --------------------------------------------------------------------------
# Bass kernel
# --------------------------------------------------------------------------
def _build_bass(a2a=True, reps=1):
    import concourse.bacc as bacc
    import concourse.mybir as mybir
    from concourse.tile import TileContext

    fp32 = mybir.dt.float32
    bf16 = mybir.dt.bfloat16
    nc = bacc.Bacc("TRN2", target_bir_lowering=False, debug=False,
                   num_devices=_NCORES)
    xc = nc.declare_dram_parameter("xc", [_SZ, _RPC], bf16, isOutput=False)
    amT = nc.declare_dram_parameter("amT", [_SZ, _SZ], bf16, isOutput=False)
    annT = nc.declare_dram_parameter("annT", [_SZ, _SZ], bf16, isOutput=False)
    cout = nc.declare_dram_parameter("cout", [_RPC, _SZ], bf16, isOutput=True)

    w_send = nc.dram_tensor("w_send", [_NCORES, _RPC, _RPC], bf16)
    w_recv = nc.dram_tensor("w_recv", [_NCORES, _RPC, _RPC], bf16)

    with TileContext(nc) as tc:
      for _rep in range(reps):  # reps>1: timing builds only (slope method)
        # ---------- phase 1: ZT[v, u] = sum_r xc[r, v] * amT[r, u] ----------
        # xc resident in SBUF as [128, (kt, v)]; amT streamed in u panels.
        with (
            tc.tile_pool(name="xcp", bufs=1) as xc_pool,
            tc.tile_pool(name="am", bufs=2) as am_pool,
            tc.tile_pool(name="ps1", bufs=8, space="PSUM") as ps1_pool,
            tc.tile_pool(name="ev1", bufs=8) as ev1_pool,
        ):
            xcs = xc_pool.tile([128, _KT * _RPC], bf16)  # 4 MB
            for q in range(4):  # split load so matmuls start early
                nc.sync.dma_start(
                    out=xcs[:].rearrange("p (kt v) -> p kt v", kt=_KT)
                    [:, q * 8:(q + 1) * 8, :],
                    in_=xc[q * 1024:(q + 1) * 1024, :]
                    .rearrange("(kt p) v -> p kt v", p=128))
            for up in range(_SZ // _UP):
                am = am_pool.tile([128, _KT * _UP], bf16, tag="am")  # 8 MB
                for q in range(4):
                    nc.sync.dma_start(
                        out=am[:].rearrange("p (kt u) -> p kt u", kt=_KT)
                        [:, q * 8:(q + 1) * 8, :],
                        in_=amT[q * 1024:(q + 1) * 1024,
                                up * _UP:(up + 1) * _UP]
                        .rearrange("(kt p) u -> p kt u", p=128))
                for vt in range(_RPC // 128):
                    for uh in range(_UP // 512):
                        ps = ps1_pool.tile([128, 512], fp32, tag="ps")
                        for kt in range(_KT):
                            nc.tensor.matmul(
                                ps[:],
                                xcs[:, kt * _RPC + vt * 128:
                                       kt * _RPC + vt * 128 + 128],
                                am[:, kt * _UP + uh * 512:
                                      kt * _UP + (uh + 1) * 512],
                                start=(kt == 0), stop=(kt == _KT - 1))
                        ev = ev1_pool.tile([128, 512], bf16, tag="ev")
                        nc.vector.tensor_copy(ev[:], ps[:])
                        j = (up * _UP + uh * 512) // _RPC  # dest core
                        nc.sync.dma_start(
                            out=w_send[j, vt * 128:(vt + 1) * 128, :],
                            in_=ev[:])

        # ---------- exchange ----------
        if a2a:
            nc.gpsimd.collective_compute(
                "AllToAll",
                mybir.AluOpType.bypass,
                ins=[w_send[:]],
                outs=[w_recv[:]],
                replica_groups=[list(range(_NCORES))],
            )
        else:  # timing-sim variant: same bytes moved, no collective
            nc.sync.dma_start(out=w_recv[:], in_=w_send[:])

        # ---------- phase 2: C[u, v] = sum_c ZT[c, u] * annT[c, v] ----------
        # w_recv resident as [128, (kt, u)]: global c = kt*128 + p with
        # kt = j*4 + s, i.e. w_recv[j, s*128+p, u]; annT streamed in v panels.
        with (
            tc.tile_pool(name="wr", bufs=1) as wr_pool,
            tc.tile_pool(name="an", bufs=2) as an_pool,
            tc.tile_pool(name="ps2", bufs=8, space="PSUM") as ps2_pool,
            tc.tile_pool(name="ev2", bufs=8) as ev2_pool,
        ):
            wr = wr_pool.tile([128, _KT * _RPC], bf16)  # 4 MB
            for j in range(_NCORES):  # per-source-block so use can start early
                nc.sync.dma_start(
                    out=wr[:].rearrange("p (j s u) -> p j s u",
                                        j=_NCORES, s=4)[:, j, :, :],
                    in_=w_recv[j].rearrange("(s p) u -> p s u", p=128))
            for vp in range(_SZ // _UP):
                an = an_pool.tile([128, _KT * _UP], bf16, tag="an")  # 8 MB
                for q in range(4):
                    nc.sync.dma_start(
                        out=an[:].rearrange("p (kt v) -> p kt v", kt=_KT)
                        [:, q * 8:(q + 1) * 8, :],
                        in_=annT[q * 1024:(q + 1) * 1024,
                                 vp * _UP:(vp + 1) * _UP]
                        .rearrange("(kt p) v -> p kt v", p=128))
                for ut in range(_RPC // 128):
                    for vh in range(_UP // 512):
                        ps = ps2_pool.tile([128, 512], fp32, tag="ps2")
                        for kt in range(_KT):
                            nc.tensor.matmul(
                                ps[:],
                                wr[:, kt * _RPC + ut * 128:
                                      kt * _RPC + ut * 128 + 128],
                                an[:, kt * _UP + vh * 512:
                                      kt * _UP + (vh + 1) * 512],
                                start=(kt == 0), stop=(kt == _KT - 1))
                        ev = ev2_pool.tile([128, 512], bf16, tag="ev2")
                        nc.vector.tensor_copy(ev[:], ps[:])
                        nc.sync.dma_start(
                            out=cout[ut * 128:(ut + 1) * 128,
                                     vp * _UP + vh * 512:
                                     vp * _UP + (vh + 1) * 512],
                            in_=ev[:])

    nc.compile()
    return nc


# --------------------------------------------------------------------------
# PJRT SPMD runner (compile once, run many)
# --------------------------------------------------------------------------
def _build_runner(nc, n_cores):
    import jax
    import jax.numpy as jnp
    from jax.sharding import Mesh, PartitionSpec as P, NamedSharding
    from jax.experimental.shard_map import shard_map
    import concourse.mybir as mybir
    from concourse import bass2jax
    from concourse.bass2jax import _bass_exec_p, partition_id_tensor

    bass2jax.install_neuronx_cc_hook()
    partition_name = (nc.partition_id_tensor.name
                      if nc.partition_id_tensor else None)

    # shardings per bass parameter (default: stacked along axis 0 per core)
    param_spec = {
        "xc": P(None, "core"),                   # column shard
        "amT": P(),                              # replicated
        "annT": P(),                             # replicated
    }

    in_names, out_names, out_avals = [], [], []
    for alloc in nc.m.functions[0].allocations:
        if not isinstance(alloc, mybir.MemoryLocationSet):
            continue
        name = alloc.memorylocations[0].name
        if alloc.kind == "ExternalInput":
            if name != partition_name:
                in_names.append(name)
        elif alloc.kind == "ExternalOutput":
            shape = tuple(alloc.tensor_shape)
            dtype = mybir.dt.np(alloc.dtype)
            out_names.append(name)
            out_avals.append(jax.core.ShapedArray(shape, dtype))
    n_outs = len(out_avals)
    in_names_all = list(in_names) + out_names
    if partition_name is not None:
        in_names_all = in_names_all + [partition_name]

    def _body(*args):
        operands = list(args)
        if partition_name is not None:
            operands.append(partition_id_tensor())
        outs = _bass_exec_p.bind(
            *operands,
            out_avals=tuple(out_avals),
            in_names=tuple(in_names_all),
            out_names=tuple(out_names),
            lowering_input_output_aliases=(),
            sim_require_finite=True,
            sim_require_nnan=True,
            nc=nc,
        )
        return tuple(outs)

    devices = jax.devices()[:n_cores]
    mesh = Mesh(np.asarray(devices), ("core",))
    in_specs = tuple(param_spec.get(nm, P("core")) for nm in in_names)
    out_sharding_specs = (P("core"),) * n_outs
    sharded = jax.jit(
        shard_map(_body, mesh=mesh,
                  in_specs=in_specs + out_sharding_specs,
                  out_specs=out_sharding_specs,
                  check_rep=False),
        keep_unused=True)

    rep_shard = NamedSharding(mesh, P())
    out_shard = NamedSharding(mesh, P("core"))
    _dev_cache = {}

    # The "output" operands of the bass_exec custom call are placeholders:
    # the NEFF's result buffers are the custom call's results, so these
    # operands are never consumed. Build them once and reuse every call -
    # one PJRT dispatch per kernel invocation.
    _zero_shapes = [(n_cores * a.shape[0], *a.shape[1:]) for a in out_avals]
    _zero_dtypes = [a.dtype for a in out_avals]
    _make_zeros = jax.jit(
        lambda: tuple(jnp.zeros(s, d)
                      for s, d in zip(_zero_shapes, _zero_dtypes)),
        out_shardings=(out_shard,) * len(_zero_shapes))
    _zeros_cache = []

    def _zeros():
        if not _zeros_cache:
            import jax as _jax
            z = _make_zeros()
            _jax.block_until_ready(z)
            _zeros_cache.append(z)
        return _zeros_cache[0]

    def _put(name, arr):
        import jax as _jax
        spec = param_spec.get(name, P("core"))
        return _jax.device_put(arr, NamedSharding(mesh, spec))

    def run(in_map, cache_names=(), block=True):
        """in_map: full global arrays keyed by bass param name."""
        import jax as _jax
        concat_in = []
        for name in in_names:
            if name in cache_names and name in _dev_cache:
                concat_in.append(_dev_cache[name])
                continue
            darr = _put(name, in_map[name])
            if name in cache_names:
                _jax.block_until_ready(darr)
                _dev_cache[name] = darr
            concat_in.append(darr)
        raw = sharded(*concat_in, *_zeros())
        if block:
            _jax.block_until_ready(raw)
        return raw[0] if n_outs == 1 else raw

    def bench(L):
        """Dispatch L back-to-back executions on cached inputs, block once.
        Returns elapsed wall seconds. Device executions serialize, so
        (bench(L2)-bench(L1))/(L2-L1) isolates per-call HW exec time from
        the constant dispatch overhead."""
        import time as _time
        import jax as _jax
        concat_in = [_dev_cache[name] for name in in_names]
        z = _zeros()
        t0 = _time.perf_counter()
        outs = []
        for _ in range(L):
            outs.append(sharded(*concat_in, *z))
        _jax.block_until_ready(outs)
        return _time.perf_counter() - t0

    run.dev_cache = _dev_cache
    run.bench = bench
    run.mesh = mesh
    return run


# --------------------------------------------------------------------------
# host-side tables
# --------------------------------------------------------------------------
def _tables(expkM, expkN):
    import ml_dtypes
    key = (expkM.tobytes(), expkN.tobytes())
    cached = _state.get("tables")
    if cached is not None and cached[0] == key:
        return cached[1], cached[2]
    run = _state.get("run")
    if run is not None:
        run.dev_cache.clear()
    n = _SZ
    i = np.arange(n)
    pm = np.where(i < (n + 1) // 2, 2 * i, 2 * (n - i) - 1)
    pinv = np.empty(n, dtype=np.int64)
    pinv[pm] = i
    # Cp[j, v] = cos(2pi * pinv[j] * v / n)
    ang = (2.0 * np.pi / n) * np.outer(pinv.astype(np.float64),
                                       i.astype(np.float64))
    Cp = np.cos(ang)
    Sp = np.sin(ang)
    eMr = expkM[:, 0].astype(np.float64)
    eMi = expkM[:, 1].astype(np.float64)
    eNr = expkN[:, 0].astype(np.float64)
    eNi = expkN[:, 1].astype(np.float64)
    bf16 = ml_dtypes.bfloat16
    annT = np.ascontiguousarray(
        (2.0 * (Cp * eNr[None, :] + Sp * eNi[None, :])).astype(bf16))
    amT = np.ascontiguousarray(
        (0.5 * (Cp * eMr[None, :] + Sp * eMi[None, :])).astype(bf16))
    _state["tables"] = (key, annT, amT)
    return annT, amT


def kernel(x, expkM, expkN, M, N):
    import ml_dtypes
    x = np.asarray(x, dtype=np.float32)
    expkM = np.asarray(expkM, dtype=np.float32)
    expkN = np.asarray(expkN, dtype=np.float32)
    assert x.shape == (_SZ, _SZ)

    annT, amT = _tables(expkM, expkN)
    if "run" not in _state:
        _state["run"] = _build_runner(_build_bass(), _NCORES)
    run = _state["run"]

    xb = x.astype(ml_dtypes.bfloat16)
    out = run({"xc": xb, "annT": annT, "amT": amT},
              cache_names=("annT", "amT"))
    return np.asarray(out).astype(np.float32)


# revision 21
# speedup vs baseline: 4.7349x; 1.2930x over previous
"""2D DCT-II (4096x4096) on 8 Trainium2 NeuronCores (axon/PJRT SPMD).

Math: the reference computes C = A_M @ x @ A_N^T where the Makhoul even-odd
permutation is folded into dense tables built from the expk inputs.  Folding
reconstitutes the *standard* DCT-II matrix in natural input order:
  A_M[u, m] = 0.5*cos(pi*u*(2m+1)/(2N)),  A_N^T[c, v] = 2*cos(pi*v*(2c+1)/(2N))
which obeys the classic even-odd split: even (odd) output rows are symmetric
(antisymmetric) under m -> N-1-m.  So with mirror butterflies
  e[m] = x[m] + x[N-1-m],  o[m] = x[m] - x[N-1-m]   (m < N/2)
each dense 4096-point transform becomes two independent 2048-point GEMMs
against the even/odd column halves of the original tables - half the PE work
and half the table traffic of the direct form.

Distribution (8 cores), with no transposes anywhere (host or device):
  core k holds x[:, cols_k].
  phase 1: row butterflies (mirror partner obtained losslessly via a PE
           matmul with the antidiagonal identity J), then
           ZT_even = e^T @ g2T, ZT_odd = o^T @ g4T  ->  Z^T[cols_k, :] in
           even/odd-of-u order; written as 8 [512, 512] blocks where block j
           holds the k'-ranges whose true u rows land on core j
           (slots [0:256) = even u, [256:512) = odd u).
  AllToAll: block j of core k -> core j; core k then holds Z^T[:, rows_k]
           (u in slot order) with the c dimension natural.
  phase 2: column butterflies on c (same J trick), then
           C_even_v = eZ^T @ h2T, C_odd_v = oZ^T @ h4T; the final evacuation
           interleaves even/odd v via stride-2 DVE writes and lands on
           stride-2 row slices of cout, undoing the slot order for free.
Host: x uploads column-sharded (bf16), C returns row-sharded (bf16) as one
global jax array - zero host-side reshuffling.  Tables upload once
(replicated, 32 MB total) and stay cached on device.

Everything is bf16 except PSUM accumulation (fp32).  Measured end-to-end
rel err ~5e-3 vs the f64 reference (gate is 2e-2).
"""
import numpy as np

_NCORES = 8
_SZ = 4096
_H = _SZ // 2           # 2048: contraction length after the butterfly
_RPC = _SZ // _NCORES   # 512 rows/cols per core
_KT = _H // 128         # 16 contraction tiles

_state = {}


# --------------------------------------------------------------------------
# Bass kernel
# --------------------------------------------------------------------------
def _build_bass(a2a=True, reps=1):
    import concourse.bacc as bacc
    import concourse.mybir as mybir
    from concourse.tile import TileContext

    fp32 = mybir.dt.float32
    bf16 = mybir.dt.bfloat16
    add = mybir.AluOpType.add
    sub = mybir.AluOpType.subtract
    mult = mybir.AluOpType.mult
    nc = bacc.Bacc("TRN2", target_bir_lowering=False, debug=False,
                   num_devices=_NCORES)
    xc = nc.declare_dram_parameter("xc", [_SZ, _RPC], bf16, isOutput=False)
    # phase-1 tables, pre-tiled: g*[panel, p, kt*1024 + u] = gT[kt*128+p,
    # panel*1024 + u] with gT = amT[:2048, parity::2]
    g2 = nc.declare_dram_parameter("g2", [2, 128, _KT * 1024], bf16,
                                   isOutput=False)
    g4 = nc.declare_dram_parameter("g4", [2, 128, _KT * 1024], bf16,
                                   isOutput=False)
    # phase-2 tables, pre-tiled in 512-wide panels of annT[:2048, parity::2]
    h2 = nc.declare_dram_parameter("h2", [4, 128, _KT * 512], bf16,
                                   isOutput=False)
    h4 = nc.declare_dram_parameter("h4", [4, 128, _KT * 512], bf16,
                                   isOutput=False)
    jrev = nc.declare_dram_parameter("jrev", [128, 128], bf16, isOutput=False)
    cout = nc.declare_dram_parameter("cout", [_RPC, _SZ], bf16, isOutput=True)

    w_send = nc.dram_tensor("w_send", [_NCORES, _RPC, _RPC], bf16)
    w_recv = nc.dram_tensor("w_recv", [_NCORES, _RPC, _RPC], bf16)

    with TileContext(nc) as tc:
      for _rep in range(reps):  # reps>1: timing builds only (slope method)
        # ------------- phase 1: butterfly + ZT = [e;o]^T @ [g2;g4] --------
        with (
            tc.tile_pool(name="xcp", bufs=1) as xc_pool,
            tc.tile_pool(name="eo", bufs=1) as eo_pool,
            tc.tile_pool(name="jp", bufs=1) as j_pool,
            tc.tile_pool(name="gp", bufs=2) as g_pool,
            tc.tile_pool(name="psj", bufs=2, space="PSUM") as psj_pool,
            tc.tile_pool(name="ps1", bufs=6, space="PSUM") as ps1_pool,
            tc.tile_pool(name="ev1", bufs=8) as ev1_pool,
        ):
            jt = j_pool.tile([128, 128], bf16)
            nc.sync.dma_start(out=jt[:], in_=jrev[:])
            xcs = xc_pool.tile([128, 2 * _KT * _RPC], bf16)  # 4 MB
            for q in (0, 3, 1, 2):  # mirror-pair order: butterfly starts
                nc.sync.dma_start(    # after the first two quarter-loads
                    out=xcs[:].rearrange("p (kt v) -> p kt v", kt=2 * _KT)
                    [:, q * 8:(q + 1) * 8, :],
                    in_=xc[q * 1024:(q + 1) * 1024, :]
                    .rearrange("(kt p) v -> p kt v", p=128))
            eT = eo_pool.tile([128, _KT * _RPC], bf16)  # 2 MB
            oT = eo_pool.tile([128, _KT * _RPC], bf16)  # 2 MB
            for kt in range(_KT):
                mir = 2 * _KT - 1 - kt
                pj = psj_pool.tile([128, _RPC], fp32, tag="pj")
                nc.tensor.matmul(
                    pj[:], jt[:],
                    xcs[:, mir * _RPC:(mir + 1) * _RPC],
                    start=True, stop=True)
                nc.vector.scalar_tensor_tensor(
                    out=eT[:, kt * _RPC:(kt + 1) * _RPC],
                    in0=xcs[:, kt * _RPC:(kt + 1) * _RPC],
                    scalar=1.0, in1=pj[:], op0=mult, op1=add)
                nc.vector.scalar_tensor_tensor(
                    out=oT[:, kt * _RPC:(kt + 1) * _RPC],
                    in0=xcs[:, kt * _RPC:(kt + 1) * _RPC],
                    scalar=1.0, in1=pj[:], op0=mult, op1=sub)
            for tab in range(2):  # 0: even u rows (g2,e)  1: odd (g4,o)
                src = eT if tab == 0 else oT
                gparam = g2 if tab == 0 else g4
                slot0 = 0 if tab == 0 else 256
                for panel in range(2):  # k' panels of 1024
                    g = g_pool.tile([128, _KT * 1024], bf16, tag="g")  # 4 MB
                    for q in range(2):
                        nc.sync.dma_start(
                            out=g[:].rearrange("p (kt u) -> p kt u", kt=_KT)
                            [:, q * 8:(q + 1) * 8, :],
                            in_=gparam[panel, :, q * 8 * 1024:
                                       (q + 1) * 8 * 1024]
                            .rearrange("p (kt u) -> p kt u", kt=8))
                    for vt in range(4):
                        for uh in range(2):
                            ps = ps1_pool.tile([128, 512], fp32, tag="ps")
                            for kt in range(_KT):
                                nc.tensor.matmul(
                                    ps[:],
                                    src[:, kt * _RPC + vt * 128:
                                           kt * _RPC + vt * 128 + 128],
                                    g[:, kt * 1024 + uh * 512:
                                         kt * 1024 + (uh + 1) * 512],
                                    start=(kt == 0), stop=(kt == _KT - 1))
                            ev = ev1_pool.tile([128, 512], bf16, tag="ev")
                            nc.vector.tensor_copy(ev[:], ps[:])
                            q_abs = panel * 2 + uh
                            nc.sync.dma_start(
                                out=w_send[2 * q_abs, vt * 128:(vt + 1) * 128,
                                           slot0:slot0 + 256],
                                in_=ev[:, :256])
                            nc.sync.dma_start(
                                out=w_send[2 * q_abs + 1,
                                           vt * 128:(vt + 1) * 128,
                                           slot0:slot0 + 256],
                                in_=ev[:, 256:])

        # ---------- exchange ----------
        if a2a:
            nc.gpsimd.collective_compute(
                "AllToAll",
                mybir.AluOpType.bypass,
                ins=[w_send[:]],
                outs=[w_recv[:]],
                replica_groups=[list(range(_NCORES))],
            )
        else:  # timing-sim variant: same bytes moved, no collective
            nc.sync.dma_start(out=w_recv[:], in_=w_send[:])

        # ------------- phase 2: butterfly on c + C = [eZ;oZ]^T @ [h2;h4] --
        with (
            tc.tile_pool(name="wrp", bufs=1) as wr_pool,
            tc.tile_pool(name="eo2", bufs=1) as eo2_pool,
            tc.tile_pool(name="jp2", bufs=1) as j2_pool,
            tc.tile_pool(name="hp", bufs=4) as h_pool,
            tc.tile_pool(name="psj2", bufs=2, space="PSUM") as psj2_pool,
            tc.tile_pool(name="psE", bufs=3, space="PSUM") as psE_pool,
            tc.tile_pool(name="psO", bufs=3, space="PSUM") as psO_pool,
            tc.tile_pool(name="ev2", bufs=4) as ev2_pool,
        ):
            jt2 = j2_pool.tile([128, 128], bf16)
            nc.sync.dma_start(out=jt2[:], in_=jrev[:])
            wr = wr_pool.tile([128, 2 * _KT * _RPC], bf16)  # 4 MB
            for j in (0, 7, 1, 6, 2, 5, 3, 4):  # mirror-pair order: the
                nc.sync.dma_start(              # butterfly starts after two
                    out=wr[:].rearrange("p (j s u) -> p j s u",
                                        j=_NCORES, s=4)[:, j, :, :],
                    in_=w_recv[j].rearrange("(s p) u -> p s u", p=128))
            eZ = eo2_pool.tile([128, _KT * _RPC], bf16)  # 2 MB
            oZ = eo2_pool.tile([128, _KT * _RPC], bf16)  # 2 MB
            for kt in range(_KT):
                mir = 2 * _KT - 1 - kt
                pj = psj2_pool.tile([128, _RPC], fp32, tag="pj2")
                nc.tensor.matmul(
                    pj[:], jt2[:],
                    wr[:, mir * _RPC:(mir + 1) * _RPC],
                    start=True, stop=True)
                nc.vector.scalar_tensor_tensor(
                    out=eZ[:, kt * _RPC:(kt + 1) * _RPC],
                    in0=wr[:, kt * _RPC:(kt + 1) * _RPC],
                    scalar=1.0, in1=pj[:], op0=mult, op1=add)
                nc.vector.scalar_tensor_tensor(
                    out=oZ[:, kt * _RPC:(kt + 1) * _RPC],
                    in0=wr[:, kt * _RPC:(kt + 1) * _RPC],
                    scalar=1.0, in1=pj[:], op0=mult, op1=sub)
            for panel in range(4):  # k panels of 512
                hc2 = h_pool.tile([128, _KT * 512], bf16, tag="h2")  # 2 MB
                hc4 = h_pool.tile([128, _KT * 512], bf16, tag="h4")  # 2 MB
                nc.sync.dma_start(out=hc2[:], in_=h2[panel])
                nc.sync.dma_start(out=hc4[:], in_=h4[panel])
                for ut in range(4):
                    psE = psE_pool.tile([128, 512], fp32, tag="psE")
                    for kt in range(_KT):
                        nc.tensor.matmul(
                            psE[:],
                            eZ[:, kt * _RPC + ut * 128:
                                  kt * _RPC + ut * 128 + 128],
                            hc2[:, kt * 512:(kt + 1) * 512],
                            start=(kt == 0), stop=(kt == _KT - 1))
                    psO = psO_pool.tile([128, 512], fp32, tag="psO")
                    for kt in range(_KT):
                        nc.tensor.matmul(
                            psO[:],
                            oZ[:, kt * _RPC + ut * 128:
                                  kt * _RPC + ut * 128 + 128],
                            hc4[:, kt * 512:(kt + 1) * 512],
                            start=(kt == 0), stop=(kt == _KT - 1))
                    ev = ev2_pool.tile([128, 1024], bf16, tag="ev2")
                    evs = ev[:].rearrange("p (k two) -> p two k", two=2)
                    nc.vector.tensor_copy(evs[:, 0, :], psE[:])
                    nc.vector.tensor_copy(evs[:, 1, :], psO[:])
                    # u-slot tile -> stride-2 row slice of cout
                    parity, urow = (0, ut) if ut < 2 else (1, ut - 2)
                    nc.sync.dma_start(
                        out=cout[:].rearrange("(u two) v -> two u v", two=2)
                        [parity, urow * 128:(urow + 1) * 128,
                         panel * 1024:(panel + 1) * 1024],
                        in_=ev[:])

    nc.compile()
    return nc


# --------------------------------------------------------------------------
# PJRT SPMD runner (compile once, run many)
# --------------------------------------------------------------------------
def _build_runner(nc, n_cores):
    import jax
    import jax.numpy as jnp
    from jax.sharding import Mesh, PartitionSpec as P, NamedSharding
    from jax.experimental.shard_map import shard_map
    import concourse.mybir as mybir
    from concourse import bass2jax
    from concourse.bass2jax import _bass_exec_p, partition_id_tensor

    bass2jax.install_neuronx_cc_hook()
    partition_name = (nc.partition_id_tensor.name
                      if nc.partition_id_tensor else None)

    # shardings per bass parameter (default: stacked along axis 0 per core)
    param_spec = {
        "xc": P(None, "core"),                   # column shard
        "g2": P(), "g4": P(), "h2": P(), "h4": P(), "jrev": P(),
        "amT": P(), "annT": P(),
    }

    in_names, out_names, out_avals = [], [], []
    for alloc in nc.m.functions[0].allocations:
        if not isinstance(alloc, mybir.MemoryLocationSet):
            continue
        name = alloc.memorylocations[0].name
        if alloc.kind == "ExternalInput":
            if name != partition_name:
                in_names.append(name)
        elif alloc.kind == "ExternalOutput":
            shape = tuple(alloc.tensor_shape)
            dtype = mybir.dt.np(alloc.dtype)
            out_names.append(name)
            out_avals.append(jax.core.ShapedArray(shape, dtype))
    n_outs = len(out_avals)
    in_names_all = list(in_names) + out_names
    if partition_name is not None:
        in_names_all = in_names_all + [partition_name]

    def _body(*args):
        operands = list(args)
        if partition_name is not None:
            operands.append(partition_id_tensor())
        outs = _bass_exec_p.bind(
            *operands,
            out_avals=tuple(out_avals),
            in_names=tuple(in_names_all),
            out_names=tuple(out_names),
            lowering_input_output_aliases=(),
            sim_require_finite=True,
            sim_require_nnan=True,
            nc=nc,
        )
        return tuple(outs)

    devices = jax.devices()[:n_cores]
    mesh = Mesh(np.asarray(devices), ("core",))
    in_specs = tuple(param_spec.get(nm, P("core")) for nm in in_names)
    out_sharding_specs = (P("core"),) * n_outs
    sharded = jax.jit(
        shard_map(_body, mesh=mesh,
                  in_specs=in_specs + out_sharding_specs,
                  out_specs=out_sharding_specs,
                  check_rep=False),
        keep_unused=True)

    out_shard = NamedSharding(mesh, P("core"))
    _dev_cache = {}

    # The "output" operands of the bass_exec custom call are placeholders:
    # the NEFF's result buffers are the custom call's results, so these
    # operands are never consumed. Build them once and reuse every call -
    # one PJRT dispatch per kernel invocation.
    _zero_shapes = [(n_cores * a.shape[0], *a.shape[1:]) for a in out_avals]
    _zero_dtypes = [a.dtype for a in out_avals]
    _make_zeros = jax.jit(
        lambda: tuple(jnp.zeros(s, d)
                      for s, d in zip(_zero_shapes, _zero_dtypes)),
        out_shardings=(out_shard,) * len(_zero_shapes))
    _zeros_cache = []

    def _zeros():
        if not _zeros_cache:
            import jax as _jax
            z = _make_zeros()
            _jax.block_until_ready(z)
            _zeros_cache.append(z)
        return _zeros_cache[0]

    def _put(name, arr):
        import jax as _jax
        spec = param_spec.get(name, P("core"))
        return _jax.device_put(arr, NamedSharding(mesh, spec))

    def run(in_map, cache_names=(), block=True):
        """in_map: full global arrays keyed by bass param name."""
        import jax as _jax
        concat_in = []
        for name in in_names:
            if name in cache_names and name in _dev_cache:
                concat_in.append(_dev_cache[name])
                continue
            darr = _put(name, in_map[name])
            if name in cache_names:
                _jax.block_until_ready(darr)
                _dev_cache[name] = darr
            concat_in.append(darr)
        raw = sharded(*concat_in, *_zeros())
        if block:
            _jax.block_until_ready(raw)
        return raw[0] if n_outs == 1 else raw

    def bench(L):
        """Dispatch L back-to-back executions on cached inputs, block once.
        Returns elapsed wall seconds."""
        import time as _time
        import jax as _jax
        concat_in = [_dev_cache[name] for name in in_names]
        z = _zeros()
        t0 = _time.perf_counter()
        outs = []
        for _ in range(L):
            outs.append(sharded(*concat_in, *z))
        _jax.block_until_ready(outs)
        return _time.perf_counter() - t0

    run.dev_cache = _dev_cache
    run.bench = bench
    run.mesh = mesh
    return run


# --------------------------------------------------------------------------
# host-side tables
# --------------------------------------------------------------------------
def _tables(expkM, expkN):
    import ml_dtypes
    key = (expkM.tobytes(), expkN.tobytes())
    cached = _state.get("tables")
    if cached is not None and cached[0] == key:
        return cached[1]
    run = _state.get("run")
    if run is not None:
        run.dev_cache.clear()
    bf16 = ml_dtypes.bfloat16
    n = _SZ
    i = np.arange(n)
    pm = np.where(i < (n + 1) // 2, 2 * i, 2 * (n - i) - 1)
    pinv = np.empty(n, dtype=np.int64)
    pinv[pm] = i
    # Cp[j, v] = cos(2pi * pinv[j] * v / n); with the permutation folded these
    # are the standard DCT-II tables in natural input order (see module doc).
    ang = (2.0 * np.pi / n) * np.outer(pinv.astype(np.float64),
                                       i.astype(np.float64))
    Cp = np.cos(ang)
    Sp = np.sin(ang)
    eMr = expkM[:, 0].astype(np.float64)
    eMi = expkM[:, 1].astype(np.float64)
    eNr = expkN[:, 0].astype(np.float64)
    eNi = expkN[:, 1].astype(np.float64)
    annT = (2.0 * (Cp * eNr[None, :] + Sp * eNi[None, :])).astype(bf16)
    amT = (0.5 * (Cp * eMr[None, :] + Sp * eMi[None, :])).astype(bf16)

    def tile_g(t):  # [2048, 2048] -> [2 panels, 128, 16*1024]
        return np.ascontiguousarray(
            t.reshape(_KT, 128, 2, 1024).transpose(2, 1, 0, 3)
            .reshape(2, 128, _KT * 1024))

    def tile_h(t):  # [2048, 2048] -> [4 panels, 128, 16*512]
        return np.ascontiguousarray(
            t.reshape(_KT, 128, 4, 512).transpose(2, 1, 0, 3)
            .reshape(4, 128, _KT * 512))

    tabs = {
        "g2": tile_g(amT[:_H, 0::2]),
        "g4": tile_g(amT[:_H, 1::2]),
        "h2": tile_h(annT[:_H, 0::2]),
        "h4": tile_h(annT[:_H, 1::2]),
        "jrev": np.ascontiguousarray(np.eye(128, dtype=bf16)[::-1]),
    }
    _state["tables"] = (key, tabs)
    return tabs


def kernel(x, expkM, expkN, M, N):
    import ml_dtypes
    x = np.asarray(x, dtype=np.float32)
    expkM = np.asarray(expkM, dtype=np.float32)
    expkN = np.asarray(expkN, dtype=np.float32)
    assert x.shape == (_SZ, _SZ)

    tabs = _tables(expkM, expkN)
    if "run" not in _state:
        _state["run"] = _build_runner(_build_bass(), _NCORES)
    run = _state["run"]

    ins = dict(tabs)
    ins["xc"] = x.astype(ml_dtypes.bfloat16)
    out = run(ins, cache_names=("g2", "g4", "h2", "h4", "jrev"))
    return np.asarray(out).astype(np.float32)
